# revision 9
# baseline (speedup 1.0000x reference)
"""Trainium2 Bass kernel: additive (Bahdanau-style) attention.

Reference math (B=16, Lq=Lc=H=256):
    qp  = query @ Wq.T                  (B, Lq, H)
    cp  = context @ Wc.T                (B, Lc, H)
    x   = qp[:,:,None,:] + cp[:,None,:,:] + w_bias     (B, Lq, Lc, H)
    score = leaky_relu(x) @ v           (B, Lq, Lc)
    score = where(mask==0, -inf, score)
    attn = softmax(score, -1); attn_output = attn @ context
    returns (attn_output, attn)

Device strategy (8 NeuronCores, data-parallel over batch, 2 batches/core):
  leaky(x) = s*x + (1-s)*relu(x), s=0.01:
    - relu part: for each (q, htile): X = relu(cpT + qpbT[:,q]) (128h, 256c)
      on VectorE (fused tensor_scalar add+max, bf16 4x-ish) and ScalarE
      (activation Relu with per-partition bias, bf16 out ~2x), split ~61/39.
      Reduced over h by TensorE 32-column-group matmuls whose stationary is a
      sliding one-hot tile (0.99*v (x) e_{j%32}^T), accumulating q's score row
      onto PSUM partition j of a (128, 256) S tile directly.
    - linear part + mask: two K=1 rank-1 matmuls into the same accumulation
      group: S += outer(0.01*vq, ones) + outer(ones, 0.01*vc + maskbias).
  All big transposes (weights, q, c, attn) are DMA xbar transposes in bf16 so
  the TensorEngine only runs the reduction + projections.
  softmax: reduce_max (negated) -> Exp activation with bias=-max and
  accum_out row-sum -> reciprocal -> scale.
"""

import numpy as np
from contextlib import ExitStack

import concourse.bass as bass
import concourse.mybir as mybir
import concourse.tile as tile
from concourse import bacc
from concourse.bass_utils import run_bass_kernel_spmd
from concourse.masks import make_identity

F32 = mybir.dt.float32
BF16 = mybir.dt.bfloat16
I32 = mybir.dt.int32
AF = mybir.ActivationFunctionType
OP = mybir.AluOpType
AX = mybir.AxisListType

B, L, H = 16, 256, 256
NCORES = 8
BL = B // NCORES          # batches per core
P = 128                   # partitions
HT = H // P               # h tiles (contraction side)
QT = L // P               # q tiles
CT = L // P               # c tiles
SLOPE = 0.01
ACT_PAT = frozenset({1, 3, 6, 8, 11})   # 5/13 of relu ops go to ScalarE


def _build_body(ctx, tc):
    nc = tc.nc
    q_d = nc.declare_dram_parameter("query", [BL, L, H], F32, isOutput=False)
    c_d = nc.declare_dram_parameter("context", [BL, L, H], F32, isOutput=False)
    m_d = nc.declare_dram_parameter("mask", [BL, L], I32, isOutput=False)
    w_d = nc.declare_dram_parameter("w_weight", [H, 2 * H], F32, isOutput=False)
    b_d = nc.declare_dram_parameter("w_bias", [H], F32, isOutput=False)
    v_d = nc.declare_dram_parameter("score_weight", [1, H], F32, isOutput=False)
    ao_d = nc.declare_dram_parameter("attn_output", [BL, L, H], F32, isOutput=True)
    at_d = nc.declare_dram_parameter("attn", [BL, L, L], F32, isOutput=True)

    consts = ctx.enter_context(tc.tile_pool(name="consts", bufs=1))
    wpool = ctx.enter_context(tc.tile_pool(name="wpool", bufs=1))
    bpool = ctx.enter_context(tc.tile_pool(name="bpool", bufs=4))
    xpool = ctx.enter_context(tc.tile_pool(name="xpool", bufs=10))
    spool = ctx.enter_context(tc.tile_pool(name="spool", bufs=4))
    psA = ctx.enter_context(tc.tile_pool(name="psA", bufs=3, space="PSUM"))
    psB = ctx.enter_context(tc.tile_pool(name="psB", bufs=4, space="PSUM"))

    # ---------------- constants / weights (once) ----------------
    ident = consts.tile([P, P], F32)
    make_identity(nc, ident)
    ones_row = consts.tile([1, L], F32)
    nc.vector.memset(ones_row, 1.0)

    wsb = []
    for r in range(HT):  # rows r*P:(r+1)*P of w_weight (h_out)
        t = wpool.tile([P, 2 * H], F32, tag=f"wsb{r}")
        nc.sync.dma_start(out=t, in_=w_d[r * P:(r + 1) * P, :])
        wsb.append(t)
    wsb16 = []
    for r in range(HT):
        t = wpool.tile([P, 2 * H], BF16, tag=f"wsb16{r}", name=f"wsb16{r}")
        nc.scalar.copy(out=t, in_=wsb[r])
        wsb16.append(t)

    # transposed bf16 weights: wqT16[ki]/wcT16[ki] = (h_in ki on partitions,
    # h_out on free), via DMA xbar transposes
    wqT16 = [wpool.tile([P, H], BF16, tag=f"wqT16{k}", name=f"wqT16{k}") for k in range(HT)]
    wcT16 = [wpool.tile([P, H], BF16, tag=f"wcT16{k}", name=f"wcT16{k}") for k in range(HT)]
    for ki in range(HT):
        for r in range(HT):
            for dst, coff in ((wqT16, 0), (wcT16, H)):
                nc.sync.dma_start_transpose(
                    dst[ki][:, r * P:(r + 1) * P],
                    wsb16[r][:, coff + ki * P: coff + (ki + 1) * P])

    vrow = wpool.tile([1, H], F32, tag="vrow")
    nc.sync.dma_start(out=vrow, in_=v_d[0:1, :])
    brow = wpool.tile([1, H], F32, tag="brow")
    nc.sync.dma_start(out=brow, in_=b_d[None, :])

    # v and bias as per-partition columns (tiny PE transposes)
    vcolf, bcol, vcol16, w99 = [], [], [], []
    for ht in range(HT):
        pv = psB.tile([P, 1], F32, tag="ps")
        nc.tensor.transpose(pv, vrow[0:1, ht * P:(ht + 1) * P], ident[0:1, 0:1])
        t = wpool.tile([P, 1], F32, tag=f"vcolf{ht}")
        nc.vector.tensor_copy(out=t, in_=pv)
        vcolf.append(t)

        pb = psB.tile([P, 1], F32, tag="ps")
        nc.tensor.transpose(pb, brow[0:1, ht * P:(ht + 1) * P], ident[0:1, 0:1])
        tb = wpool.tile([P, 1], F32, tag=f"bcol{ht}")
        nc.vector.tensor_copy(out=tb, in_=pb)
        bcol.append(tb)

        t16 = wpool.tile([P, 1], BF16, tag=f"vcol16{ht}")
        nc.vector.tensor_copy(out=t16, in_=pv)
        vcol16.append(t16)

        # sliding one-hot stationaries for 32-column-group matmuls.
        # we[:, 32] = 0.99*v (even j%32), wo[:, 33] = same (odd j%32): every
        # 32-wide slice starts 4-byte aligned.
        we = wpool.tile([P, 64], BF16, tag=f"w99e{ht}", name=f"w99e{ht}")
        nc.vector.memset(we, 0.0)
        nc.vector.tensor_scalar(out=we[:, 32:33], in0=pv, scalar1=1.0 - SLOPE,
                                scalar2=None, op0=OP.mult)
        wo = wpool.tile([P, 66], BF16, tag=f"w99o{ht}", name=f"w99o{ht}")
        nc.vector.memset(wo, 0.0)
        nc.vector.tensor_scalar(out=wo[:, 33:34], in0=pv, scalar1=1.0 - SLOPE,
                                scalar2=None, op0=OP.mult)
        w99.append((we, wo))

    # ---------------- per batch ----------------
    for b in range(BL):
        qsb = [bpool.tile([P, H], F32, tag=f"qsb{i}", name=f"qsb{i}") for i in range(QT)]
        csb = [bpool.tile([P, H], F32, tag=f"csb{i}", name=f"csb{i}") for i in range(CT)]
        for qi in range(QT):
            nc.sync.dma_start(out=qsb[qi], in_=q_d[b, qi * P:(qi + 1) * P, :])
        for ci in range(CT):
            nc.sync.dma_start(out=csb[ci], in_=c_d[b, ci * P:(ci + 1) * P, :])
        qsb16 = [bpool.tile([P, H], BF16, tag=f"qsb16{i}", name=f"qsb16{i}") for i in range(QT)]
        csb16 = [bpool.tile([P, H], BF16, tag=f"csb16{i}", name=f"csb16{i}") for i in range(CT)]
        for qi in range(QT):
            nc.scalar.copy(out=qsb16[qi], in_=qsb[qi])
        for ci in range(CT):
            nc.scalar.copy(out=csb16[ci], in_=csb[ci])

        mrow_i = bpool.tile([1, L], I32, tag="mrow_i")
        nc.sync.dma_start(out=mrow_i, in_=m_d[b:b + 1, :])
        mrow_f = bpool.tile([1, L], F32, tag="mrow_f")
        nc.vector.tensor_copy(out=mrow_f, in_=mrow_i)
        maskb = bpool.tile([1, L], F32, tag="maskb")
        nc.vector.tensor_scalar(out=maskb, in0=mrow_f, scalar1=-1.0, scalar2=1e30,
                                op0=OP.add, op1=OP.mult)

        # qT16/cT16 = (h on partitions, q/c on free) via DMA xbar transposes
        qT16 = [bpool.tile([P, L], BF16, tag=f"qT16{i}", name=f"qT16{i}") for i in range(HT)]
        cT16 = [bpool.tile([P, L], BF16, tag=f"cT16{i}", name=f"cT16{i}") for i in range(HT)]
        for src, dst in ((qsb16, qT16), (csb16, cT16)):
            for ti in range(QT):
                for hi in range(HT):
                    nc.sync.dma_start_transpose(
                        dst[hi][:, ti * P:(ti + 1) * P],
                        src[ti][:, hi * P:(hi + 1) * P])

        # projections (bf16 matmuls, fp32 accumulate)
        qpbT = [bpool.tile([P, L], F32, tag=f"qpbT{i}", name=f"qpbT{i}") for i in range(HT)]
        cpT16 = [bpool.tile([P, L], BF16, tag=f"cpT16{i}", name=f"cpT16{i}") for i in range(HT)]
        for ho in range(HT):
            pq = psB.tile([P, L], F32, tag="ps")
            for ki in range(HT):
                nc.tensor.matmul(pq, wqT16[ki][:, ho * P:(ho + 1) * P], qT16[ki],
                                 start=(ki == 0), stop=(ki == HT - 1))
            nc.vector.tensor_scalar(out=qpbT[ho], in0=pq, scalar1=bcol[ho],
                                    scalar2=None, op0=OP.add)
            pc = psB.tile([P, L], F32, tag="ps")
            for ki in range(HT):
                nc.tensor.matmul(pc, wcT16[ki][:, ho * P:(ho + 1) * P], cT16[ki],
                                 start=(ki == 0), stop=(ki == HT - 1))
            nc.scalar.copy(out=cpT16[ho], in_=pc)

        # vq / vc rows via M=1 matvecs (land on psum partition 0)
        pvq = psB.tile([1, L], F32, tag="ps")
        for ht in range(HT):
            nc.tensor.matmul(pvq, vcolf[ht], qpbT[ht], start=(ht == 0), stop=(ht == HT - 1))
        pvc = psB.tile([1, L], F32, tag="ps")
        for ht in range(HT):
            nc.tensor.matmul(pvc, vcol16[ht], cpT16[ht], start=(ht == 0), stop=(ht == HT - 1))

        linq = bpool.tile([1, L], F32, tag="linq")
        nc.vector.tensor_scalar(out=linq, in0=pvq, scalar1=SLOPE,
                                scalar2=None, op0=OP.mult)
        rowvec = bpool.tile([1, L], F32, tag="rowvec")
        nc.vector.tensor_scalar(out=rowvec, in0=pvc, scalar1=SLOPE,
                                scalar2=None, op0=OP.mult)
        nc.vector.tensor_add(rowvec, rowvec, maskb)

        attnT16 = [bpool.tile([P, L], BF16, tag=f"attnT16{i}", name=f"attnT16{i}") for i in range(CT)]

        opctr = 0
        for qi in range(QT):
            sp = psA.tile([P, L], F32, tag="spsum")
            nc.tensor.matmul(sp, linq[0:1, qi * P:(qi + 1) * P], ones_row,
                             start=True, stop=False)
            nc.tensor.matmul(sp, ones_row[0:1, 0:P], rowvec,
                             start=False, stop=False)
            for j in range(P):
                q = qi * P + j
                g, jr = j // 32, j % 32
                for ht in range(HT):
                    x = xpool.tile([P, L], BF16, tag="x")
                    bias_col = qpbT[ht][:, q:q + 1]
                    if opctr % 13 in ACT_PAT:
                        nc.scalar.activation(out=x, in_=cpT16[ht], func=AF.Relu,
                                             bias=bias_col, scale=1.0)
                    else:
                        nc.vector.tensor_scalar(out=x, in0=cpT16[ht], scalar1=bias_col,
                                                scalar2=0.0, op0=OP.add, op1=OP.max)
                    opctr += 1
                    last = (j == P - 1 and ht == HT - 1)
                    we, wo = w99[ht]
                    lhsT = we[:, 32 - jr:64 - jr] if jr % 2 == 0 else wo[:, 33 - jr:65 - jr]
                    nc.tensor.matmul(sp[32 * g:32 * (g + 1), :], lhsT, x,
                                     start=False, stop=last,
                                     tile_position=(0, 32 * g),
                                     skip_group_check=True)

            # softmax over c
            mx = spool.tile([P, 1], F32, tag="mx")
            nc.vector.tensor_reduce(out=mx, in_=sp, axis=AX.X, op=OP.max, negate=True)
            pexp = spool.tile([P, L], F32, tag="pexp")
            rsum = spool.tile([P, 1], F32, tag="rsum")
            nc.scalar.activation(out=pexp, in_=sp, func=AF.Exp, bias=mx, scale=1.0,
                                 accum_out=rsum)
            rinv = spool.tile([P, 1], F32, tag="rinv")
            nc.vector.reciprocal(out=rinv, in_=rsum)
            attn_sb = spool.tile([P, L], F32, tag="attn_sb")
            nc.vector.tensor_scalar(out=attn_sb, in0=pexp, scalar1=rinv,
                                    scalar2=None, op0=OP.mult)
            nc.sync.dma_start(out=at_d[b, qi * P:(qi + 1) * P, :], in_=attn_sb)

            # attn_output = attn @ context (bf16), transposes via DMA xbar
            attn16 = spool.tile([P, L], BF16, tag="attn16")
            nc.scalar.copy(out=attn16, in_=attn_sb)
            for ci in range(CT):
                nc.sync.dma_start_transpose(
                    attnT16[ci][:, qi * P:(qi + 1) * P],
                    attn16[:, ci * P:(ci + 1) * P])
            po = psB.tile([P, H], F32, tag="ps")
            for ci in range(CT):
                nc.tensor.matmul(po, attnT16[ci][:, qi * P:(qi + 1) * P], csb16[ci],
                                 start=(ci == 0), stop=(ci == CT - 1))
            osb = spool.tile([P, H], F32, tag="osb")
            nc.scalar.copy(out=osb, in_=po)
            nc.sync.dma_start(out=ao_d[b, qi * P:(qi + 1) * P, :], in_=osb)


_NC_CACHE = {}


def build_nc():
    if "nc" in _NC_CACHE:
        return _NC_CACHE["nc"]
    nc = bacc.Bacc("TRN2", target_bir_lowering=False)
    with ExitStack() as ctx:
        tc = ctx.enter_context(tile.TileContext(nc))
        _build_body(ctx, tc)
    nc.compile()
    _NC_CACHE["nc"] = nc
    return nc


def kernel(query, context, mask, w_weight, w_bias, score_weight, _trace=False):
    query = np.ascontiguousarray(np.asarray(query, dtype=np.float32))
    context = np.ascontiguousarray(np.asarray(context, dtype=np.float32))
    mask = np.ascontiguousarray(np.asarray(mask, dtype=np.int32))
    w_weight = np.ascontiguousarray(np.asarray(w_weight, dtype=np.float32))
    w_bias = np.ascontiguousarray(np.asarray(w_bias, dtype=np.float32))
    score_weight = np.ascontiguousarray(np.asarray(score_weight, dtype=np.float32))

    nc = build_nc()
    in_maps = []
    for i in range(NCORES):
        sl = slice(i * BL, (i + 1) * BL)
        in_maps.append({
            "query": query[sl], "context": context[sl], "mask": mask[sl],
            "w_weight": w_weight, "w_bias": w_bias, "score_weight": score_weight,
        })
    res = run_bass_kernel_spmd(nc, in_maps, core_ids=list(range(NCORES)),
                               trace=_trace)
    attn_output = np.concatenate([r["attn_output"] for r in res.results], axis=0)
    attn = np.concatenate([r["attn"] for r in res.results], axis=0)
    if _trace:
        kernel.last_exec_time_ns = res.exec_time_ns
        kernel.last_results = res
    return attn_output, attn


# revision 11
# speedup vs baseline: 1.2145x; 1.2145x over previous
"""Trainium2 Bass kernel: additive (Bahdanau-style) attention.

Reference math (B=16, Lq=Lc=H=256):
    qp  = query @ Wq.T                  (B, Lq, H)
    cp  = context @ Wc.T                (B, Lc, H)
    x   = qp[:,:,None,:] + cp[:,None,:,:] + w_bias     (B, Lq, Lc, H)
    score = leaky_relu(x) @ v           (B, Lq, Lc)
    score = where(mask==0, -inf, score)
    attn = softmax(score, -1); attn_output = attn @ context
    returns (attn_output, attn)

Device strategy (8 NeuronCores, data-parallel over batch, 2 batches/core):
  leaky(x) = s*x + (1-s)*relu(x), s=0.01:
    - relu part: for each (q, htile): X = relu(cpT + qpbT[:,q]) (128h, 256c)
      on VectorE (fused tensor_scalar add+max, bf16 4x-ish) and ScalarE
      (activation Relu with per-partition bias, bf16 out ~2x), split ~61/39.
      Reduced over h by TensorE 32-column-group matmuls whose stationary is a
      sliding one-hot tile (0.99*v (x) e_{j%32}^T), accumulating q's score row
      onto PSUM partition j of a (128, 256) S tile directly.
    - linear part + mask: two K=1 rank-1 matmuls into the same accumulation
      group: S += outer(0.01*vq, ones) + outer(ones, 0.01*vc + maskbias).
  All big transposes (weights, q, c, attn) are DMA xbar transposes in bf16 so
  the TensorEngine only runs the reduction + projections.
  softmax: reduce_max (negated) -> Exp activation with bias=-max and
  accum_out row-sum -> reciprocal -> scale.
"""

import numpy as np
from contextlib import ExitStack

import concourse.bass as bass
import concourse.mybir as mybir
import concourse.tile as tile
from concourse import bacc
from concourse.bass_utils import run_bass_kernel_spmd
from concourse.masks import make_identity

F32 = mybir.dt.float32
BF16 = mybir.dt.bfloat16
I32 = mybir.dt.int32
AF = mybir.ActivationFunctionType
OP = mybir.AluOpType
AX = mybir.AxisListType

B, L, H = 16, 256, 256
NCORES = 8
BL = B // NCORES          # batches per core
P = 128                   # partitions
HT = H // P               # h tiles (contraction side)
QT = L // P               # q tiles
CT = L // P               # c tiles
SLOPE = 0.01
ACT_PAT = frozenset({1, 3, 6, 8, 11})   # 5/13 of relu ops go to ScalarE


def _build_body(ctx, tc):
    nc = tc.nc
    q_d = nc.declare_dram_parameter("query", [BL, L, H], F32, isOutput=False)
    c_d = nc.declare_dram_parameter("context", [BL, L, H], F32, isOutput=False)
    m_d = nc.declare_dram_parameter("mask", [BL, L], I32, isOutput=False)
    w_d = nc.declare_dram_parameter("w_weight", [H, 2 * H], F32, isOutput=False)
    b_d = nc.declare_dram_parameter("w_bias", [H], F32, isOutput=False)
    v_d = nc.declare_dram_parameter("score_weight", [1, H], F32, isOutput=False)
    ao_d = nc.declare_dram_parameter("attn_output", [BL, L, H], F32, isOutput=True)
    at_d = nc.declare_dram_parameter("attn", [BL, L, L], F32, isOutput=True)

    consts = ctx.enter_context(tc.tile_pool(name="consts", bufs=1))
    wpool = ctx.enter_context(tc.tile_pool(name="wpool", bufs=1))
    bpool = ctx.enter_context(tc.tile_pool(name="bpool", bufs=4))
    xpool = ctx.enter_context(tc.tile_pool(name="xpool", bufs=10))
    spool = ctx.enter_context(tc.tile_pool(name="spool", bufs=4))
    psA = ctx.enter_context(tc.tile_pool(name="psA", bufs=3, space="PSUM"))
    psB = ctx.enter_context(tc.tile_pool(name="psB", bufs=4, space="PSUM"))

    # ---------------- constants / weights (once) ----------------
    ident = consts.tile([P, P], F32)
    make_identity(nc, ident)
    ones_row = consts.tile([1, L], F32)
    nc.vector.memset(ones_row, 1.0)

    wsb = []
    for r in range(HT):  # rows r*P:(r+1)*P of w_weight (h_out)
        t = wpool.tile([P, 2 * H], F32, tag=f"wsb{r}")
        nc.sync.dma_start(out=t, in_=w_d[r * P:(r + 1) * P, :])
        wsb.append(t)
    # transposed bf16 weights: wqT16[ki]/wcT16[ki] = (h_in ki on partitions,
    # h_out on free), via PE transposes (fp32 in, bf16 out on the copy)
    wqT16 = [wpool.tile([P, H], BF16, tag=f"wqT16{k}", name=f"wqT16{k}") for k in range(HT)]
    wcT16 = [wpool.tile([P, H], BF16, tag=f"wcT16{k}", name=f"wcT16{k}") for k in range(HT)]
    for ki in range(HT):
        for r in range(HT):
            for dst, coff in ((wqT16, 0), (wcT16, H)):
                pst = psB.tile([P, P], F32, tag="ps")
                nc.tensor.transpose(pst, wsb[r][:, coff + ki * P: coff + (ki + 1) * P], ident)
                nc.scalar.copy(out=dst[ki][:, r * P:(r + 1) * P], in_=pst)

    vrow = wpool.tile([1, H], F32, tag="vrow")
    nc.sync.dma_start(out=vrow, in_=v_d[0:1, :])
    brow = wpool.tile([1, H], F32, tag="brow")
    nc.sync.dma_start(out=brow, in_=b_d[None, :])

    # v and bias as per-partition columns (tiny PE transposes)
    vcolf, bcol, vcol16, w99 = [], [], [], []
    for ht in range(HT):
        pv = psB.tile([P, 1], F32, tag="ps")
        nc.tensor.transpose(pv, vrow[0:1, ht * P:(ht + 1) * P], ident[0:1, 0:1])
        t = wpool.tile([P, 1], F32, tag=f"vcolf{ht}")
        nc.vector.tensor_copy(out=t, in_=pv)
        vcolf.append(t)

        pb = psB.tile([P, 1], F32, tag="ps")
        nc.tensor.transpose(pb, brow[0:1, ht * P:(ht + 1) * P], ident[0:1, 0:1])
        tb = wpool.tile([P, 1], F32, tag=f"bcol{ht}")
        nc.vector.tensor_copy(out=tb, in_=pb)
        bcol.append(tb)

        t16 = wpool.tile([P, 1], BF16, tag=f"vcol16{ht}")
        nc.vector.tensor_copy(out=t16, in_=pv)
        vcol16.append(t16)

        # sliding one-hot stationaries for 32-column-group matmuls.
        # we[:, 32] = 0.99*v (even j%32), wo[:, 33] = same (odd j%32): every
        # 32-wide slice starts 4-byte aligned.
        we = wpool.tile([P, 64], BF16, tag=f"w99e{ht}", name=f"w99e{ht}")
        nc.vector.memset(we, 0.0)
        nc.vector.tensor_scalar(out=we[:, 32:33], in0=pv, scalar1=1.0 - SLOPE,
                                scalar2=None, op0=OP.mult)
        wo = wpool.tile([P, 66], BF16, tag=f"w99o{ht}", name=f"w99o{ht}")
        nc.vector.memset(wo, 0.0)
        nc.vector.tensor_scalar(out=wo[:, 33:34], in0=pv, scalar1=1.0 - SLOPE,
                                scalar2=None, op0=OP.mult)
        w99.append((we, wo))

    # ---------------- per batch ----------------
    for b in range(BL):
        qsb = [bpool.tile([P, H], F32, tag=f"qsb{i}", name=f"qsb{i}") for i in range(QT)]
        csb = [bpool.tile([P, H], F32, tag=f"csb{i}", name=f"csb{i}") for i in range(CT)]
        for qi in range(QT):
            nc.sync.dma_start(out=qsb[qi], in_=q_d[b, qi * P:(qi + 1) * P, :])
        for ci in range(CT):
            nc.sync.dma_start(out=csb[ci], in_=c_d[b, ci * P:(ci + 1) * P, :])
        csb16 = [bpool.tile([P, H], BF16, tag=f"csb16{i}", name=f"csb16{i}") for i in range(CT)]
        for ci in range(CT):
            nc.scalar.copy(out=csb16[ci], in_=csb[ci])

        mrow_i = bpool.tile([1, L], I32, tag="mrow_i")
        nc.sync.dma_start(out=mrow_i, in_=m_d[b:b + 1, :])
        mrow_f = bpool.tile([1, L], F32, tag="mrow_f")
        nc.vector.tensor_copy(out=mrow_f, in_=mrow_i)
        maskb = bpool.tile([1, L], F32, tag="maskb")
        nc.vector.tensor_scalar(out=maskb, in0=mrow_f, scalar1=-1.0, scalar2=1e30,
                                op0=OP.add, op1=OP.mult)

        # qT16/cT16 = (h on partitions, q/c on free) via PE transposes (fp32 in,
        # bf16 cast on the psum evacuation, split across DVE/ACT)
        qT16 = [bpool.tile([P, L], BF16, tag=f"qT16{i}", name=f"qT16{i}") for i in range(HT)]
        cT16 = [bpool.tile([P, L], BF16, tag=f"cT16{i}", name=f"cT16{i}") for i in range(HT)]
        cpi = 0
        for srcl, dst in ((qsb, qT16), (csb, cT16)):
            for ti in range(QT):
                for hi in range(HT):
                    pst = psB.tile([P, P], F32, tag="ps")
                    nc.tensor.transpose(pst, srcl[ti][:, hi * P:(hi + 1) * P], ident)
                    eng = nc.vector if cpi % 2 == 0 else nc.scalar
                    if eng is nc.vector:
                        nc.vector.tensor_copy(out=dst[hi][:, ti * P:(ti + 1) * P], in_=pst)
                    else:
                        nc.scalar.copy(out=dst[hi][:, ti * P:(ti + 1) * P], in_=pst)
                    cpi += 1

        # projections (bf16 matmuls, fp32 accumulate)
        qpbT = [bpool.tile([P, L], F32, tag=f"qpbT{i}", name=f"qpbT{i}") for i in range(HT)]
        cpT16 = [bpool.tile([P, L], BF16, tag=f"cpT16{i}", name=f"cpT16{i}") for i in range(HT)]
        for ho in range(HT):
            pq = psB.tile([P, L], F32, tag="ps")
            for ki in range(HT):
                nc.tensor.matmul(pq, wqT16[ki][:, ho * P:(ho + 1) * P], qT16[ki],
                                 start=(ki == 0), stop=(ki == HT - 1))
            nc.vector.tensor_scalar(out=qpbT[ho], in0=pq, scalar1=bcol[ho],
                                    scalar2=None, op0=OP.add)
            pc = psB.tile([P, L], F32, tag="ps")
            for ki in range(HT):
                nc.tensor.matmul(pc, wcT16[ki][:, ho * P:(ho + 1) * P], cT16[ki],
                                 start=(ki == 0), stop=(ki == HT - 1))
            nc.scalar.copy(out=cpT16[ho], in_=pc)

        # vq / vc rows via M=1 matvecs (land on psum partition 0)
        pvq = psB.tile([1, L], F32, tag="ps")
        for ht in range(HT):
            nc.tensor.matmul(pvq, vcolf[ht], qpbT[ht], start=(ht == 0), stop=(ht == HT - 1))
        pvc = psB.tile([1, L], F32, tag="ps")
        for ht in range(HT):
            nc.tensor.matmul(pvc, vcol16[ht], cpT16[ht], start=(ht == 0), stop=(ht == HT - 1))

        linq = bpool.tile([1, L], F32, tag="linq")
        nc.vector.tensor_scalar(out=linq, in0=pvq, scalar1=SLOPE,
                                scalar2=None, op0=OP.mult)
        rowvec = bpool.tile([1, L], F32, tag="rowvec")
        nc.vector.tensor_scalar(out=rowvec, in0=pvc, scalar1=SLOPE,
                                scalar2=None, op0=OP.mult)
        nc.vector.tensor_add(rowvec, rowvec, maskb)

        attnT16 = [bpool.tile([P, L], BF16, tag=f"attnT16{i}", name=f"attnT16{i}") for i in range(CT)]

        opctr = 0
        for qi in range(QT):
            sp = psA.tile([P, L], F32, tag="spsum")
            # full-width rank-1 matmuls first: start=True must cover all 128
            # partitions so every row's has_written bit is freshly set.
            nc.tensor.matmul(sp, linq[0:1, qi * P:(qi + 1) * P], ones_row,
                             start=True, stop=False)
            nc.tensor.matmul(sp, ones_row[0:1, 0:P], rowvec,
                             start=False, stop=False)
            for j in range(P):
                q = qi * P + j
                g, jr = j // 32, j % 32
                for ht in range(HT):
                    x = xpool.tile([P, L], BF16, tag="x")
                    bias_col = qpbT[ht][:, q:q + 1]
                    if opctr % 13 in ACT_PAT:
                        nc.scalar.activation(out=x, in_=cpT16[ht], func=AF.Relu,
                                             bias=bias_col, scale=1.0)
                    else:
                        nc.vector.tensor_scalar(out=x, in0=cpT16[ht], scalar1=bias_col,
                                                scalar2=0.0, op0=OP.add, op1=OP.max)
                    opctr += 1
                    last = (j == P - 1 and ht == HT - 1)
                    we, wo = w99[ht]
                    lhsT = we[:, 32 - jr:64 - jr] if jr % 2 == 0 else wo[:, 33 - jr:65 - jr]
                    nc.tensor.matmul(sp[32 * g:32 * (g + 1), :], lhsT, x,
                                     start=False, stop=last,
                                     tile_position=(0, 32 * g),
                                     skip_group_check=True)

            # softmax over c
            mx = spool.tile([P, 1], F32, tag="mx")
            nc.vector.tensor_reduce(out=mx, in_=sp, axis=AX.X, op=OP.max, negate=True)
            pexp = spool.tile([P, L], F32, tag="pexp")
            rsum = spool.tile([P, 1], F32, tag="rsum")
            nc.scalar.activation(out=pexp, in_=sp, func=AF.Exp, bias=mx, scale=1.0,
                                 accum_out=rsum)
            rinv = spool.tile([P, 1], F32, tag="rinv")
            nc.vector.reciprocal(out=rinv, in_=rsum)
            attn_sb = spool.tile([P, L], F32, tag="attn_sb")
            nc.vector.tensor_scalar(out=attn_sb, in0=pexp, scalar1=rinv,
                                    scalar2=None, op0=OP.mult)
            nc.sync.dma_start(out=at_d[b, qi * P:(qi + 1) * P, :], in_=attn_sb)

            # attn_output = attn @ context (bf16), transposes on PE
            for ci in range(CT):
                pst = psB.tile([P, P], F32, tag="ps")
                nc.tensor.transpose(pst, attn_sb[:, ci * P:(ci + 1) * P], ident)
                nc.vector.tensor_copy(out=attnT16[ci][:, qi * P:(qi + 1) * P], in_=pst)
            po = psB.tile([P, H], F32, tag="ps")
            for ci in range(CT):
                nc.tensor.matmul(po, attnT16[ci][:, qi * P:(qi + 1) * P], csb16[ci],
                                 start=(ci == 0), stop=(ci == CT - 1))
            osb = spool.tile([P, H], F32, tag="osb")
            nc.scalar.copy(out=osb, in_=po)
            nc.sync.dma_start(out=ao_d[b, qi * P:(qi + 1) * P, :], in_=osb)


_NC_CACHE = {}


def build_nc():
    if "nc" in _NC_CACHE:
        return _NC_CACHE["nc"]
    nc = bacc.Bacc("TRN2", target_bir_lowering=False)
    with ExitStack() as ctx:
        tc = ctx.enter_context(tile.TileContext(nc))
        _build_body(ctx, tc)
    nc.compile()
    _NC_CACHE["nc"] = nc
    return nc


def kernel(query, context, mask, w_weight, w_bias, score_weight, _trace=False):
    query = np.ascontiguousarray(np.asarray(query, dtype=np.float32))
    context = np.ascontiguousarray(np.asarray(context, dtype=np.float32))
    mask = np.ascontiguousarray(np.asarray(mask, dtype=np.int32))
    w_weight = np.ascontiguousarray(np.asarray(w_weight, dtype=np.float32))
    w_bias = np.ascontiguousarray(np.asarray(w_bias, dtype=np.float32))
    score_weight = np.ascontiguousarray(np.asarray(score_weight, dtype=np.float32))

    nc = build_nc()
    in_maps = []
    for i in range(NCORES):
        sl = slice(i * BL, (i + 1) * BL)
        in_maps.append({
            "query": query[sl], "context": context[sl], "mask": mask[sl],
            "w_weight": w_weight, "w_bias": w_bias, "score_weight": score_weight,
        })
    res = run_bass_kernel_spmd(nc, in_maps, core_ids=list(range(NCORES)),
                               trace=_trace)
    attn_output = np.concatenate([r["attn_output"] for r in res.results], axis=0)
    attn = np.concatenate([r["attn"] for r in res.results], axis=0)
    if _trace:
        kernel.last_exec_time_ns = res.exec_time_ns
        kernel.last_results = res
    return attn_output, attn


# revision 12
# speedup vs baseline: 1.2707x; 1.0462x over previous
"""Trainium2 Bass kernel: additive (Bahdanau-style) attention.

Reference math (B=16, Lq=Lc=H=256):
    qp  = query @ Wq.T                  (B, Lq, H)
    cp  = context @ Wc.T                (B, Lc, H)
    x   = qp[:,:,None,:] + cp[:,None,:,:] + w_bias     (B, Lq, Lc, H)
    score = leaky_relu(x) @ v           (B, Lq, Lc)
    score = where(mask==0, -inf, score)
    attn = softmax(score, -1); attn_output = attn @ context
    returns (attn_output, attn)

Device strategy (8 NeuronCores, data-parallel over batch, 2 batches/core):
  leaky(x) = s*x + (1-s)*relu(x), s=0.01:
    - relu part: for each (q, htile): X = relu(cpT + qpbT[:,q]) (128h, 256c)
      on VectorE (fused tensor_scalar add+max, bf16 4x-ish) and ScalarE
      (activation Relu with per-partition bias, bf16 out ~2x), split ~61/39.
      Reduced over h by TensorE 32-column-group matmuls whose stationary is a
      sliding one-hot tile (0.99*v (x) e_{j%32}^T), accumulating q's score row
      onto PSUM partition j of a (128, 256) S tile directly.
    - linear part + mask: two K=1 rank-1 matmuls into the same accumulation
      group: S += outer(0.01*vq, ones) + outer(ones, 0.01*vc + maskbias).
  All big transposes (weights, q, c, attn) are DMA xbar transposes in bf16 so
  the TensorEngine only runs the reduction + projections.
  softmax: reduce_max (negated) -> Exp activation with bias=-max and
  accum_out row-sum -> reciprocal -> scale.
"""

import numpy as np
from contextlib import ExitStack

import concourse.bass as bass
import concourse.mybir as mybir
import concourse.tile as tile
from concourse import bacc
from concourse.bass_utils import run_bass_kernel_spmd
from concourse.masks import make_identity

F32 = mybir.dt.float32
BF16 = mybir.dt.bfloat16
I32 = mybir.dt.int32
AF = mybir.ActivationFunctionType
OP = mybir.AluOpType
AX = mybir.AxisListType

B, L, H = 16, 256, 256
NCORES = 8
BL = B // NCORES          # batches per core
P = 128                   # partitions
HT = H // P               # h tiles (contraction side)
QT = L // P               # q tiles
CT = L // P               # c tiles
SLOPE = 0.01
ACT_PAT = frozenset({2, 6, 10})   # of relu ops mod 12 go to ScalarE


def _build_body(ctx, tc):
    nc = tc.nc
    q_d = nc.declare_dram_parameter("query", [BL, L, H], F32, isOutput=False)
    c_d = nc.declare_dram_parameter("context", [BL, L, H], F32, isOutput=False)
    m_d = nc.declare_dram_parameter("mask", [BL, L], I32, isOutput=False)
    w_d = nc.declare_dram_parameter("w_weight", [H, 2 * H], F32, isOutput=False)
    b_d = nc.declare_dram_parameter("w_bias", [H], F32, isOutput=False)
    v_d = nc.declare_dram_parameter("score_weight", [1, H], F32, isOutput=False)
    ao_d = nc.declare_dram_parameter("attn_output", [BL, L, H], F32, isOutput=True)
    at_d = nc.declare_dram_parameter("attn", [BL, L, L], F32, isOutput=True)

    consts = ctx.enter_context(tc.tile_pool(name="consts", bufs=1))
    wpool = ctx.enter_context(tc.tile_pool(name="wpool", bufs=1))
    bpool = ctx.enter_context(tc.tile_pool(name="bpool", bufs=4))
    xpool = ctx.enter_context(tc.tile_pool(name="xpool", bufs=10))
    spool = ctx.enter_context(tc.tile_pool(name="spool", bufs=4))
    psA = ctx.enter_context(tc.tile_pool(name="psA", bufs=3, space="PSUM"))
    psB = ctx.enter_context(tc.tile_pool(name="psB", bufs=4, space="PSUM"))

    # ---------------- constants / weights (once) ----------------
    ident = consts.tile([P, P], F32)
    make_identity(nc, ident)
    ones_row = consts.tile([1, L], F32)
    nc.vector.memset(ones_row, 1.0)

    wsb = []
    for r in range(HT):  # rows r*P:(r+1)*P of w_weight (h_out)
        t = wpool.tile([P, 2 * H], F32, tag=f"wsb{r}")
        nc.sync.dma_start(out=t, in_=w_d[r * P:(r + 1) * P, :])
        wsb.append(t)
    # transposed bf16 weights: wqT16[ki]/wcT16[ki] = (h_in ki on partitions,
    # h_out on free), via PE transposes (fp32 in, bf16 out on the copy)
    wqT16 = [wpool.tile([P, H], BF16, tag=f"wqT16{k}", name=f"wqT16{k}") for k in range(HT)]
    wcT16 = [wpool.tile([P, H], BF16, tag=f"wcT16{k}", name=f"wcT16{k}") for k in range(HT)]
    for ki in range(HT):
        for r in range(HT):
            for dst, coff in ((wqT16, 0), (wcT16, H)):
                pst = psB.tile([P, P], F32, tag="ps")
                nc.tensor.transpose(pst, wsb[r][:, coff + ki * P: coff + (ki + 1) * P], ident)
                nc.scalar.copy(out=dst[ki][:, r * P:(r + 1) * P], in_=pst)

    vrow = wpool.tile([1, H], F32, tag="vrow")
    nc.sync.dma_start(out=vrow, in_=v_d[0:1, :])
    brow = wpool.tile([1, H], F32, tag="brow")
    nc.sync.dma_start(out=brow, in_=b_d[None, :])

    # v and bias as per-partition columns (tiny PE transposes)
    vcolf, bcol, vcol16, w99 = [], [], [], []
    for ht in range(HT):
        pv = psB.tile([P, 1], F32, tag="ps")
        nc.tensor.transpose(pv, vrow[0:1, ht * P:(ht + 1) * P], ident[0:1, 0:1])
        t = wpool.tile([P, 1], F32, tag=f"vcolf{ht}")
        nc.vector.tensor_copy(out=t, in_=pv)
        vcolf.append(t)

        pb = psB.tile([P, 1], F32, tag="ps")
        nc.tensor.transpose(pb, brow[0:1, ht * P:(ht + 1) * P], ident[0:1, 0:1])
        tb = wpool.tile([P, 1], F32, tag=f"bcol{ht}")
        nc.vector.tensor_copy(out=tb, in_=pb)
        bcol.append(tb)

        t16 = wpool.tile([P, 1], BF16, tag=f"vcol16{ht}")
        nc.vector.tensor_copy(out=t16, in_=pv)
        vcol16.append(t16)

        # sliding one-hot stationaries for 32-column-group matmuls.
        # we[:, 32] = 0.99*v (even j%32), wo[:, 33] = same (odd j%32): every
        # 32-wide slice starts 4-byte aligned.
        we = wpool.tile([P, 64], BF16, tag=f"w99e{ht}", name=f"w99e{ht}")
        nc.vector.memset(we, 0.0)
        nc.vector.tensor_scalar(out=we[:, 32:33], in0=pv, scalar1=1.0 - SLOPE,
                                scalar2=None, op0=OP.mult)
        wo = wpool.tile([P, 66], BF16, tag=f"w99o{ht}", name=f"w99o{ht}")
        nc.vector.memset(wo, 0.0)
        nc.vector.tensor_scalar(out=wo[:, 33:34], in0=pv, scalar1=1.0 - SLOPE,
                                scalar2=None, op0=OP.mult)
        w99.append((we, wo))

    # ---------------- prep phase: loads/transposes/projections, both batches --
    prep = []
    for b in range(BL):
        qsb = [bpool.tile([P, H], F32, tag=f"qsb{i}", name=f"qsb{i}") for i in range(QT)]
        csb = [bpool.tile([P, H], F32, tag=f"csb{i}", name=f"csb{i}") for i in range(CT)]
        for qi in range(QT):
            nc.sync.dma_start(out=qsb[qi], in_=q_d[b, qi * P:(qi + 1) * P, :])
        for ci in range(CT):
            nc.sync.dma_start(out=csb[ci], in_=c_d[b, ci * P:(ci + 1) * P, :])
        csb16 = [bpool.tile([P, H], BF16, tag=f"csb16{i}", name=f"csb16{i}") for i in range(CT)]
        for ci in range(CT):
            nc.scalar.copy(out=csb16[ci], in_=csb[ci])

        mrow_i = bpool.tile([1, L], I32, tag="mrow_i")
        nc.sync.dma_start(out=mrow_i, in_=m_d[b:b + 1, :])
        mrow_f = bpool.tile([1, L], F32, tag="mrow_f")
        nc.vector.tensor_copy(out=mrow_f, in_=mrow_i)
        maskb = bpool.tile([1, L], F32, tag="maskb")
        nc.vector.tensor_scalar(out=maskb, in0=mrow_f, scalar1=-1.0, scalar2=1e30,
                                op0=OP.add, op1=OP.mult)

        # qT16/cT16 = (h on partitions, q/c on free) via PE transposes (fp32 in,
        # bf16 cast on the psum evacuation, split across DVE/ACT)
        qT16 = [bpool.tile([P, L], BF16, tag=f"qT16{i}", name=f"qT16{i}") for i in range(HT)]
        cT16 = [bpool.tile([P, L], BF16, tag=f"cT16{i}", name=f"cT16{i}") for i in range(HT)]
        cpi = 0
        for srcl, dst in ((qsb, qT16), (csb, cT16)):
            for ti in range(QT):
                for hi in range(HT):
                    pst = psB.tile([P, P], F32, tag="ps")
                    nc.tensor.transpose(pst, srcl[ti][:, hi * P:(hi + 1) * P], ident)
                    eng = nc.vector if cpi % 2 == 0 else nc.scalar
                    if eng is nc.vector:
                        nc.vector.tensor_copy(out=dst[hi][:, ti * P:(ti + 1) * P], in_=pst)
                    else:
                        nc.scalar.copy(out=dst[hi][:, ti * P:(ti + 1) * P], in_=pst)
                    cpi += 1

        # projections (bf16 matmuls, fp32 accumulate)
        qpbT = [bpool.tile([P, L], F32, tag=f"qpbT{i}", name=f"qpbT{i}") for i in range(HT)]
        cpT16 = [bpool.tile([P, L], BF16, tag=f"cpT16{i}", name=f"cpT16{i}") for i in range(HT)]
        for ho in range(HT):
            pq = psB.tile([P, L], F32, tag="ps")
            for ki in range(HT):
                nc.tensor.matmul(pq, wqT16[ki][:, ho * P:(ho + 1) * P], qT16[ki],
                                 start=(ki == 0), stop=(ki == HT - 1))
            nc.vector.tensor_scalar(out=qpbT[ho], in0=pq, scalar1=bcol[ho],
                                    scalar2=None, op0=OP.add)
            pc = psB.tile([P, L], F32, tag="ps")
            for ki in range(HT):
                nc.tensor.matmul(pc, wcT16[ki][:, ho * P:(ho + 1) * P], cT16[ki],
                                 start=(ki == 0), stop=(ki == HT - 1))
            nc.scalar.copy(out=cpT16[ho], in_=pc)

        # vq / vc rows via M=1 matvecs (land on psum partition 0)
        pvq = psB.tile([1, L], F32, tag="ps")
        for ht in range(HT):
            nc.tensor.matmul(pvq, vcolf[ht], qpbT[ht], start=(ht == 0), stop=(ht == HT - 1))
        pvc = psB.tile([1, L], F32, tag="ps")
        for ht in range(HT):
            nc.tensor.matmul(pvc, vcol16[ht], cpT16[ht], start=(ht == 0), stop=(ht == HT - 1))

        linq = bpool.tile([1, L], F32, tag="linq")
        nc.vector.tensor_scalar(out=linq, in0=pvq, scalar1=SLOPE,
                                scalar2=None, op0=OP.mult)
        rowvec = bpool.tile([1, L], F32, tag="rowvec")
        nc.vector.tensor_scalar(out=rowvec, in0=pvc, scalar1=SLOPE,
                                scalar2=None, op0=OP.mult)
        nc.vector.tensor_add(rowvec, rowvec, maskb)

        prep.append(dict(qpbT=qpbT, cpT16=cpT16, csb16=csb16, linq=linq,
                         rowvec=rowvec))

    # ---------------- main phase: score/softmax/output, both batches ----------
    opctr = 0
    for b in range(BL):
        qpbT = prep[b]["qpbT"]
        cpT16 = prep[b]["cpT16"]
        csb16 = prep[b]["csb16"]
        linq = prep[b]["linq"]
        rowvec = prep[b]["rowvec"]
        attnT16 = [bpool.tile([P, L], BF16, tag=f"attnT16{i}", name=f"attnT16{i}") for i in range(CT)]

        for qi in range(QT):
            sp = psA.tile([P, L], F32, tag="spsum")
            # full-width rank-1 matmuls first: start=True must cover all 128
            # partitions so every row's has_written bit is freshly set.
            nc.tensor.matmul(sp, linq[0:1, qi * P:(qi + 1) * P], ones_row,
                             start=True, stop=False)
            nc.tensor.matmul(sp, ones_row[0:1, 0:P], rowvec,
                             start=False, stop=False)
            for j in range(P):
                q = qi * P + j
                g, jr = j // 32, j % 32
                for ht in range(HT):
                    x = xpool.tile([P, L], BF16, tag="x")
                    bias_col = qpbT[ht][:, q:q + 1]
                    if opctr % 12 in ACT_PAT:
                        nc.scalar.activation(out=x, in_=cpT16[ht], func=AF.Relu,
                                             bias=bias_col, scale=1.0)
                    else:
                        nc.vector.tensor_scalar(out=x, in0=cpT16[ht], scalar1=bias_col,
                                                scalar2=0.0, op0=OP.add, op1=OP.max)
                    opctr += 1
                    last = (j == P - 1 and ht == HT - 1)
                    we, wo = w99[ht]
                    lhsT = we[:, 32 - jr:64 - jr] if jr % 2 == 0 else wo[:, 33 - jr:65 - jr]
                    nc.tensor.matmul(sp[32 * g:32 * (g + 1), :], lhsT, x,
                                     start=False, stop=last,
                                     tile_position=(0, 32 * g),
                                     skip_group_check=True)

            # softmax over c
            mx = spool.tile([P, 1], F32, tag="mx")
            nc.vector.tensor_reduce(out=mx, in_=sp, axis=AX.X, op=OP.max, negate=True)
            pexp = spool.tile([P, L], F32, tag="pexp")
            rsum = spool.tile([P, 1], F32, tag="rsum")
            nc.scalar.activation(out=pexp, in_=sp, func=AF.Exp, bias=mx, scale=1.0,
                                 accum_out=rsum)
            rinv = spool.tile([P, 1], F32, tag="rinv")
            nc.vector.reciprocal(out=rinv, in_=rsum)
            attn_sb = spool.tile([P, L], F32, tag="attn_sb")
            nc.vector.tensor_scalar(out=attn_sb, in0=pexp, scalar1=rinv,
                                    scalar2=None, op0=OP.mult)
            nc.sync.dma_start(out=at_d[b, qi * P:(qi + 1) * P, :], in_=attn_sb)

            # attn_output = attn @ context (bf16), transposes on PE
            for ci in range(CT):
                pst = psB.tile([P, P], F32, tag="ps")
                nc.tensor.transpose(pst, attn_sb[:, ci * P:(ci + 1) * P], ident)
                nc.scalar.copy(out=attnT16[ci][:, qi * P:(qi + 1) * P], in_=pst)
            po = psB.tile([P, H], F32, tag="ps")
            for ci in range(CT):
                nc.tensor.matmul(po, attnT16[ci][:, qi * P:(qi + 1) * P], csb16[ci],
                                 start=(ci == 0), stop=(ci == CT - 1))
            osb = spool.tile([P, H], F32, tag="osb")
            nc.scalar.copy(out=osb, in_=po)
            nc.sync.dma_start(out=ao_d[b, qi * P:(qi + 1) * P, :], in_=osb)


_NC_CACHE = {}


def build_nc():
    if "nc" in _NC_CACHE:
        return _NC_CACHE["nc"]
    nc = bacc.Bacc("TRN2", target_bir_lowering=False)
    with ExitStack() as ctx:
        tc = ctx.enter_context(tile.TileContext(nc))
        _build_body(ctx, tc)
    nc.compile()
    _NC_CACHE["nc"] = nc
    return nc


def kernel(query, context, mask, w_weight, w_bias, score_weight, _trace=False):
    query = np.ascontiguousarray(np.asarray(query, dtype=np.float32))
    context = np.ascontiguousarray(np.asarray(context, dtype=np.float32))
    mask = np.ascontiguousarray(np.asarray(mask, dtype=np.int32))
    w_weight = np.ascontiguousarray(np.asarray(w_weight, dtype=np.float32))
    w_bias = np.ascontiguousarray(np.asarray(w_bias, dtype=np.float32))
    score_weight = np.ascontiguousarray(np.asarray(score_weight, dtype=np.float32))

    nc = build_nc()
    in_maps = []
    for i in range(NCORES):
        sl = slice(i * BL, (i + 1) * BL)
        in_maps.append({
            "query": query[sl], "context": context[sl], "mask": mask[sl],
            "w_weight": w_weight, "w_bias": w_bias, "score_weight": score_weight,
        })
    res = run_bass_kernel_spmd(nc, in_maps, core_ids=list(range(NCORES)),
                               trace=_trace)
    attn_output = np.concatenate([r["attn_output"] for r in res.results], axis=0)
    attn = np.concatenate([r["attn"] for r in res.results], axis=0)
    if _trace:
        kernel.last_exec_time_ns = res.exec_time_ns
        kernel.last_results = res
    return attn_output, attn


# revision 13
# speedup vs baseline: 1.3433x; 1.0572x over previous
"""Trainium2 Bass kernel: additive (Bahdanau-style) attention.

Reference math (B=16, Lq=Lc=H=256):
    qp  = query @ Wq.T                  (B, Lq, H)
    cp  = context @ Wc.T                (B, Lc, H)
    x   = qp[:,:,None,:] + cp[:,None,:,:] + w_bias     (B, Lq, Lc, H)
    score = leaky_relu(x) @ v           (B, Lq, Lc)
    score = where(mask==0, -inf, score)
    attn = softmax(score, -1); attn_output = attn @ context
    returns (attn_output, attn)

Device strategy (8 NeuronCores, data-parallel over batch, 2 batches/core):
  leaky(x) = s*x + (1-s)*relu(x), s=0.01:
    - relu part: for each (q, htile): X = relu(cpT + qpbT[:,q]) (128h, 256c)
      on VectorE (fused tensor_scalar add+max, bf16 4x-ish) and ScalarE
      (activation Relu with per-partition bias, bf16 out ~2x), split ~61/39.
      Reduced over h by TensorE 32-column-group matmuls whose stationary is a
      sliding one-hot tile (0.99*v (x) e_{j%32}^T), accumulating q's score row
      onto PSUM partition j of a (128, 256) S tile directly.
    - linear part + mask: two K=1 rank-1 matmuls into the same accumulation
      group: S += outer(0.01*vq, ones) + outer(ones, 0.01*vc + maskbias).
  All big transposes (weights, q, c, attn) are DMA xbar transposes in bf16 so
  the TensorEngine only runs the reduction + projections.
  softmax: reduce_max (negated) -> Exp activation with bias=-max and
  accum_out row-sum -> reciprocal -> scale.
"""

import numpy as np
from contextlib import ExitStack

import concourse.bass as bass
import concourse.mybir as mybir
import concourse.tile as tile
from concourse import bacc
from concourse.bass_utils import run_bass_kernel_spmd
from concourse.masks import make_identity

F32 = mybir.dt.float32
BF16 = mybir.dt.bfloat16
I32 = mybir.dt.int32
AF = mybir.ActivationFunctionType
OP = mybir.AluOpType
AX = mybir.AxisListType

B, L, H = 16, 256, 256
NCORES = 8
BL = B // NCORES          # batches per core
P = 128                   # partitions
HT = H // P               # h tiles (contraction side)
QT = L // P               # q tiles
CT = L // P               # c tiles
SLOPE = 0.01
ACT_PAT = frozenset({2, 5, 8, 11})   # of relu ops mod 12 go to ScalarE


def _build_body(ctx, tc):
    nc = tc.nc
    q_d = nc.declare_dram_parameter("query", [BL, L, H], F32, isOutput=False)
    c_d = nc.declare_dram_parameter("context", [BL, L, H], F32, isOutput=False)
    m_d = nc.declare_dram_parameter("mask", [BL, L], I32, isOutput=False)
    w_d = nc.declare_dram_parameter("w_weight", [H, 2 * H], F32, isOutput=False)
    b_d = nc.declare_dram_parameter("w_bias", [H], F32, isOutput=False)
    v_d = nc.declare_dram_parameter("score_weight", [1, H], F32, isOutput=False)
    ao_d = nc.declare_dram_parameter("attn_output", [BL, L, H], F32, isOutput=True)
    at_d = nc.declare_dram_parameter("attn", [BL, L, L], F32, isOutput=True)

    consts = ctx.enter_context(tc.tile_pool(name="consts", bufs=1))
    wpool = ctx.enter_context(tc.tile_pool(name="wpool", bufs=1))
    bpool = ctx.enter_context(tc.tile_pool(name="bpool", bufs=4))
    xpool = ctx.enter_context(tc.tile_pool(name="xpool", bufs=12))
    spool = ctx.enter_context(tc.tile_pool(name="spool", bufs=4))
    psA = ctx.enter_context(tc.tile_pool(name="psA", bufs=3, space="PSUM"))
    psB = ctx.enter_context(tc.tile_pool(name="psB", bufs=4, space="PSUM"))

    # ---------------- constants / weights (once) ----------------
    ident = consts.tile([P, P], F32)
    make_identity(nc, ident)
    ones_row = consts.tile([1, L], F32)
    nc.vector.memset(ones_row, 1.0)

    wsb = []
    for r in range(HT):  # rows r*P:(r+1)*P of w_weight (h_out)
        t = wpool.tile([P, 2 * H], F32, tag=f"wsb{r}")
        nc.sync.dma_start(out=t, in_=w_d[r * P:(r + 1) * P, :])
        wsb.append(t)
    # transposed bf16 weights: wqT16[ki]/wcT16[ki] = (h_in ki on partitions,
    # h_out on free), via PE transposes (fp32 in, bf16 out on the copy)
    wqT16 = [wpool.tile([P, H], BF16, tag=f"wqT16{k}", name=f"wqT16{k}") for k in range(HT)]
    wcT16 = [wpool.tile([P, H], BF16, tag=f"wcT16{k}", name=f"wcT16{k}") for k in range(HT)]
    for ki in range(HT):
        for r in range(HT):
            for dst, coff in ((wqT16, 0), (wcT16, H)):
                pst = psB.tile([P, P], F32, tag="ps")
                nc.tensor.transpose(pst, wsb[r][:, coff + ki * P: coff + (ki + 1) * P], ident)
                nc.scalar.copy(out=dst[ki][:, r * P:(r + 1) * P], in_=pst)

    vrow = wpool.tile([1, H], F32, tag="vrow")
    nc.sync.dma_start(out=vrow, in_=v_d[0:1, :])
    brow = wpool.tile([1, H], F32, tag="brow")
    nc.sync.dma_start(out=brow, in_=b_d[None, :])

    # v and bias as per-partition columns (tiny PE transposes)
    vcolf, bcol, vcol16, w99 = [], [], [], []
    for ht in range(HT):
        pv = psB.tile([P, 1], F32, tag="ps")
        nc.tensor.transpose(pv, vrow[0:1, ht * P:(ht + 1) * P], ident[0:1, 0:1])
        t = wpool.tile([P, 1], F32, tag=f"vcolf{ht}")
        nc.vector.tensor_copy(out=t, in_=pv)
        vcolf.append(t)

        pb = psB.tile([P, 1], F32, tag="ps")
        nc.tensor.transpose(pb, brow[0:1, ht * P:(ht + 1) * P], ident[0:1, 0:1])
        tb = wpool.tile([P, 1], F32, tag=f"bcol{ht}")
        nc.vector.tensor_copy(out=tb, in_=pb)
        bcol.append(tb)

        t16 = wpool.tile([P, 1], BF16, tag=f"vcol16{ht}")
        nc.vector.tensor_copy(out=t16, in_=pv)
        vcol16.append(t16)

        # sliding one-hot stationaries for 32-column-group matmuls.
        # we[:, 32] = 0.99*v (even j%32), wo[:, 33] = same (odd j%32): every
        # 32-wide slice starts 4-byte aligned.
        we = wpool.tile([P, 64], BF16, tag=f"w99e{ht}", name=f"w99e{ht}")
        nc.vector.memset(we, 0.0)
        nc.vector.tensor_scalar(out=we[:, 32:33], in0=pv, scalar1=1.0 - SLOPE,
                                scalar2=None, op0=OP.mult)
        wo = wpool.tile([P, 66], BF16, tag=f"w99o{ht}", name=f"w99o{ht}")
        nc.vector.memset(wo, 0.0)
        nc.vector.tensor_scalar(out=wo[:, 33:34], in0=pv, scalar1=1.0 - SLOPE,
                                scalar2=None, op0=OP.mult)
        w99.append((we, wo))

    # ---------------- prep phase: loads/transposes/projections, both batches --
    prep = []
    for b in range(BL):
        qsb = [bpool.tile([P, H], F32, tag=f"qsb{i}", name=f"qsb{i}") for i in range(QT)]
        csb = [bpool.tile([P, H], F32, tag=f"csb{i}", name=f"csb{i}") for i in range(CT)]
        for qi in range(QT):
            nc.sync.dma_start(out=qsb[qi], in_=q_d[b, qi * P:(qi + 1) * P, :])
        for ci in range(CT):
            nc.sync.dma_start(out=csb[ci], in_=c_d[b, ci * P:(ci + 1) * P, :])
        csb16 = [bpool.tile([P, H], BF16, tag=f"csb16{i}", name=f"csb16{i}") for i in range(CT)]
        for ci in range(CT):
            nc.scalar.copy(out=csb16[ci], in_=csb[ci])

        mrow_i = bpool.tile([1, L], I32, tag="mrow_i")
        nc.sync.dma_start(out=mrow_i, in_=m_d[b:b + 1, :])
        mrow_f = bpool.tile([1, L], F32, tag="mrow_f")
        nc.vector.tensor_copy(out=mrow_f, in_=mrow_i)
        maskb = bpool.tile([1, L], F32, tag="maskb")
        nc.vector.tensor_scalar(out=maskb, in0=mrow_f, scalar1=-1.0, scalar2=1e30,
                                op0=OP.add, op1=OP.mult)

        # qT16/cT16 = (h on partitions, q/c on free) via PE transposes (fp32 in,
        # bf16 cast on the psum evacuation, split across DVE/ACT)
        qT16 = [bpool.tile([P, L], BF16, tag=f"qT16{i}", name=f"qT16{i}") for i in range(HT)]
        cT16 = [bpool.tile([P, L], BF16, tag=f"cT16{i}", name=f"cT16{i}") for i in range(HT)]
        cpi = 0
        for srcl, dst in ((qsb, qT16), (csb, cT16)):
            for ti in range(QT):
                for hi in range(HT):
                    pst = psB.tile([P, P], F32, tag="ps")
                    nc.tensor.transpose(pst, srcl[ti][:, hi * P:(hi + 1) * P], ident)
                    eng = nc.vector if cpi % 2 == 0 else nc.scalar
                    if eng is nc.vector:
                        nc.vector.tensor_copy(out=dst[hi][:, ti * P:(ti + 1) * P], in_=pst)
                    else:
                        nc.scalar.copy(out=dst[hi][:, ti * P:(ti + 1) * P], in_=pst)
                    cpi += 1

        # projections (bf16 matmuls, fp32 accumulate)
        qpbT = [bpool.tile([P, L], F32, tag=f"qpbT{i}", name=f"qpbT{i}") for i in range(HT)]
        cpT16 = [bpool.tile([P, L], BF16, tag=f"cpT16{i}", name=f"cpT16{i}") for i in range(HT)]
        for ho in range(HT):
            pq = psB.tile([P, L], F32, tag="ps")
            for ki in range(HT):
                nc.tensor.matmul(pq, wqT16[ki][:, ho * P:(ho + 1) * P], qT16[ki],
                                 start=(ki == 0), stop=(ki == HT - 1))
            nc.vector.tensor_scalar(out=qpbT[ho], in0=pq, scalar1=bcol[ho],
                                    scalar2=None, op0=OP.add)
            pc = psB.tile([P, L], F32, tag="ps")
            for ki in range(HT):
                nc.tensor.matmul(pc, wcT16[ki][:, ho * P:(ho + 1) * P], cT16[ki],
                                 start=(ki == 0), stop=(ki == HT - 1))
            nc.scalar.copy(out=cpT16[ho], in_=pc)

        # vq / vc rows via M=1 matvecs (land on psum partition 0)
        pvq = psB.tile([1, L], F32, tag="ps")
        for ht in range(HT):
            nc.tensor.matmul(pvq, vcolf[ht], qpbT[ht], start=(ht == 0), stop=(ht == HT - 1))
        pvc = psB.tile([1, L], F32, tag="ps")
        for ht in range(HT):
            nc.tensor.matmul(pvc, vcol16[ht], cpT16[ht], start=(ht == 0), stop=(ht == HT - 1))

        linq = bpool.tile([1, L], F32, tag="linq")
        nc.vector.tensor_scalar(out=linq, in0=pvq, scalar1=SLOPE,
                                scalar2=None, op0=OP.mult)
        rowvec = bpool.tile([1, L], F32, tag="rowvec")
        nc.vector.tensor_scalar(out=rowvec, in0=pvc, scalar1=SLOPE,
                                scalar2=None, op0=OP.mult)
        nc.vector.tensor_add(rowvec, rowvec, maskb)

        prep.append(dict(qpbT=qpbT, cpT16=cpT16, csb16=csb16, linq=linq,
                         rowvec=rowvec))

    # ---------------- main phase: score/softmax/output, both batches ----------
    opctr = 0
    for b in range(BL):
        qpbT = prep[b]["qpbT"]
        cpT16 = prep[b]["cpT16"]
        csb16 = prep[b]["csb16"]
        linq = prep[b]["linq"]
        rowvec = prep[b]["rowvec"]
        attnT16 = [bpool.tile([P, L], BF16, tag=f"attnT16{i}", name=f"attnT16{i}") for i in range(CT)]

        for qi in range(QT):
            sp = psA.tile([P, L], F32, tag="spsum")
            # full-width rank-1 matmuls first: start=True must cover all 128
            # partitions so every row's has_written bit is freshly set.
            nc.tensor.matmul(sp, linq[0:1, qi * P:(qi + 1) * P], ones_row,
                             start=True, stop=False)
            nc.tensor.matmul(sp, ones_row[0:1, 0:P], rowvec,
                             start=False, stop=False)
            for j in range(P):
                q = qi * P + j
                g, jr = j // 32, j % 32
                for ht in range(HT):
                    x = xpool.tile([P, L], BF16, tag="x")
                    bias_col = qpbT[ht][:, q:q + 1]
                    if opctr % 12 in ACT_PAT:
                        nc.scalar.activation(out=x, in_=cpT16[ht], func=AF.Relu,
                                             bias=bias_col, scale=1.0)
                    else:
                        nc.vector.tensor_scalar(out=x, in0=cpT16[ht], scalar1=bias_col,
                                                scalar2=0.0, op0=OP.add, op1=OP.max)
                    opctr += 1
                    last = (j == P - 1 and ht == HT - 1)
                    we, wo = w99[ht]
                    lhsT = we[:, 32 - jr:64 - jr] if jr % 2 == 0 else wo[:, 33 - jr:65 - jr]
                    nc.tensor.matmul(sp[32 * g:32 * (g + 1), :], lhsT, x,
                                     start=False, stop=last,
                                     tile_position=(0, 32 * g),
                                     skip_group_check=True)

            # softmax over c
            mx = spool.tile([P, 1], F32, tag="mx")
            nc.vector.tensor_reduce(out=mx, in_=sp, axis=AX.X, op=OP.max, negate=True)
            pexp = spool.tile([P, L], F32, tag="pexp")
            rsum = spool.tile([P, 1], F32, tag="rsum")
            nc.scalar.activation(out=pexp, in_=sp, func=AF.Exp, bias=mx, scale=1.0,
                                 accum_out=rsum)
            rinv = spool.tile([P, 1], F32, tag="rinv")
            nc.vector.reciprocal(out=rinv, in_=rsum)
            attn_sb = spool.tile([P, L], F32, tag="attn_sb")
            nc.vector.tensor_scalar(out=attn_sb, in0=pexp, scalar1=rinv,
                                    scalar2=None, op0=OP.mult)
            nc.sync.dma_start(out=at_d[b, qi * P:(qi + 1) * P, :], in_=attn_sb)

            # attn_output = attn @ context (bf16), transposes on PE
            for ci in range(CT):
                pst = psB.tile([P, P], F32, tag="ps")
                nc.tensor.transpose(pst, attn_sb[:, ci * P:(ci + 1) * P], ident)
                nc.scalar.copy(out=attnT16[ci][:, qi * P:(qi + 1) * P], in_=pst)
            po = psB.tile([P, H], F32, tag="ps")
            for ci in range(CT):
                nc.tensor.matmul(po, attnT16[ci][:, qi * P:(qi + 1) * P], csb16[ci],
                                 start=(ci == 0), stop=(ci == CT - 1))
            osb = spool.tile([P, H], F32, tag="osb")
            nc.scalar.copy(out=osb, in_=po)
            nc.sync.dma_start(out=ao_d[b, qi * P:(qi + 1) * P, :], in_=osb)


_NC_CACHE = {}


def build_nc():
    if "nc" in _NC_CACHE:
        return _NC_CACHE["nc"]
    nc = bacc.Bacc("TRN2", target_bir_lowering=False)
    with ExitStack() as ctx:
        tc = ctx.enter_context(tile.TileContext(nc))
        _build_body(ctx, tc)
    nc.compile()
    _NC_CACHE["nc"] = nc
    return nc


def kernel(query, context, mask, w_weight, w_bias, score_weight, _trace=False):
    query = np.ascontiguousarray(np.asarray(query, dtype=np.float32))
    context = np.ascontiguousarray(np.asarray(context, dtype=np.float32))
    mask = np.ascontiguousarray(np.asarray(mask, dtype=np.int32))
    w_weight = np.ascontiguousarray(np.asarray(w_weight, dtype=np.float32))
    w_bias = np.ascontiguousarray(np.asarray(w_bias, dtype=np.float32))
    score_weight = np.ascontiguousarray(np.asarray(score_weight, dtype=np.float32))

    nc = build_nc()
    in_maps = []
    for i in range(NCORES):
        sl = slice(i * BL, (i + 1) * BL)
        in_maps.append({
            "query": query[sl], "context": context[sl], "mask": mask[sl],
            "w_weight": w_weight, "w_bias": w_bias, "score_weight": score_weight,
        })
    res = run_bass_kernel_spmd(nc, in_maps, core_ids=list(range(NCORES)),
                               trace=_trace)
    attn_output = np.concatenate([r["attn_output"] for r in res.results], axis=0)
    attn = np.concatenate([r["attn"] for r in res.results], axis=0)
    if _trace:
        kernel.last_exec_time_ns = res.exec_time_ns
        kernel.last_results = res
    return attn_output, attn


# revision 15
# speedup vs baseline: 1.3694x; 1.0195x over previous
"""Trainium2 Bass kernel: additive (Bahdanau-style) attention.

Reference math (B=16, Lq=Lc=H=256):
    qp  = query @ Wq.T                  (B, Lq, H)
    cp  = context @ Wc.T                (B, Lc, H)
    x   = qp[:,:,None,:] + cp[:,None,:,:] + w_bias     (B, Lq, Lc, H)
    score = leaky_relu(x) @ v           (B, Lq, Lc)
    score = where(mask==0, -inf, score)
    attn = softmax(score, -1); attn_output = attn @ context
    returns (attn_output, attn)

Device strategy (8 NeuronCores, data-parallel over batch, 2 batches/core):
  leaky(x) = s*x + (1-s)*relu(x), s=0.01:
    - relu part: for each (q, htile): X = relu(cpT + qpbT[:,q]) (128h, 256c)
      on VectorE (fused tensor_scalar add+max, bf16 4x-ish) and ScalarE
      (activation Relu with per-partition bias, bf16 out ~2x), split ~61/39.
      Reduced over h by TensorE 32-column-group matmuls whose stationary is a
      sliding one-hot tile (0.99*v (x) e_{j%32}^T), accumulating q's score row
      onto PSUM partition j of a (128, 256) S tile directly.
    - linear part + mask: two K=1 rank-1 matmuls into the same accumulation
      group: S += outer(0.01*vq, ones) + outer(ones, 0.01*vc + maskbias).
  All big transposes (weights, q, c, attn) are DMA xbar transposes in bf16 so
  the TensorEngine only runs the reduction + projections.
  softmax: reduce_max (negated) -> Exp activation with bias=-max and
  accum_out row-sum -> reciprocal -> scale.
"""

import numpy as np
from contextlib import ExitStack

import concourse.bass as bass
import concourse.mybir as mybir
import concourse.tile as tile
from concourse import bacc
from concourse.bass_utils import run_bass_kernel_spmd
from concourse.masks import make_identity

F32 = mybir.dt.float32
BF16 = mybir.dt.bfloat16
I32 = mybir.dt.int32
AF = mybir.ActivationFunctionType
OP = mybir.AluOpType
AX = mybir.AxisListType

B, L, H = 16, 256, 256
NCORES = 8
BL = B // NCORES          # batches per core
P = 128                   # partitions
HT = H // P               # h tiles (contraction side)
QT = L // P               # q tiles
CT = L // P               # c tiles
SLOPE = 0.01
ACT_PAT = frozenset({2, 5, 8, 11})   # of relu ops mod 12 go to ScalarE


def _build_body(ctx, tc):
    nc = tc.nc
    q_d = nc.declare_dram_parameter("query", [BL, L, H], F32, isOutput=False)
    c_d = nc.declare_dram_parameter("context", [BL, L, H], F32, isOutput=False)
    m_d = nc.declare_dram_parameter("mask", [BL, L], I32, isOutput=False)
    w_d = nc.declare_dram_parameter("w_weight", [H, 2 * H], F32, isOutput=False)
    b_d = nc.declare_dram_parameter("w_bias", [H], F32, isOutput=False)
    v_d = nc.declare_dram_parameter("score_weight", [1, H], F32, isOutput=False)
    ao_d = nc.declare_dram_parameter("attn_output", [BL, L, H], F32, isOutput=True)
    at_d = nc.declare_dram_parameter("attn", [BL, L, L], F32, isOutput=True)

    consts = ctx.enter_context(tc.tile_pool(name="consts", bufs=1))
    wpool = ctx.enter_context(tc.tile_pool(name="wpool", bufs=1))
    bpool = ctx.enter_context(tc.tile_pool(name="bpool", bufs=4))
    xpool = ctx.enter_context(tc.tile_pool(name="xpool", bufs=12))
    spool = ctx.enter_context(tc.tile_pool(name="spool", bufs=4))
    psA = ctx.enter_context(tc.tile_pool(name="psA", bufs=3, space="PSUM"))
    psB = ctx.enter_context(tc.tile_pool(name="psB", bufs=4, space="PSUM"))

    # ---------------- constants / weights (once) ----------------
    ident = consts.tile([P, P], F32)
    make_identity(nc, ident)
    ones_row = consts.tile([1, L], F32)
    nc.vector.memset(ones_row, 1.0)

    wsb = []
    for r in range(HT):  # rows r*P:(r+1)*P of w_weight (h_out)
        t = wpool.tile([P, 2 * H], F32, tag=f"wsb{r}")
        nc.sync.dma_start(out=t, in_=w_d[r * P:(r + 1) * P, :])
        wsb.append(t)
    # transposed bf16 weights: wqT16[ki]/wcT16[ki] = (h_in ki on partitions,
    # h_out on free), via PE transposes (fp32 in, bf16 out on the copy)
    wqT16 = [wpool.tile([P, H], BF16, tag=f"wqT16{k}", name=f"wqT16{k}") for k in range(HT)]
    wcT16 = [wpool.tile([P, H], BF16, tag=f"wcT16{k}", name=f"wcT16{k}") for k in range(HT)]
    for ki in range(HT):
        for r in range(HT):
            for dst, coff in ((wqT16, 0), (wcT16, H)):
                pst = psB.tile([P, P], F32, tag="ps", name="ps_t")
                nc.tensor.transpose(pst, wsb[r][:, coff + ki * P: coff + (ki + 1) * P], ident)
                nc.scalar.copy(out=dst[ki][:, r * P:(r + 1) * P], in_=pst)

    vrow = wpool.tile([1, H], F32, tag="vrow")
    nc.sync.dma_start(out=vrow, in_=v_d[0:1, :])
    brow = wpool.tile([1, H], F32, tag="brow")
    nc.sync.dma_start(out=brow, in_=b_d[None, :])

    # v and bias as per-partition columns (tiny PE transposes)
    vcolf, bcol, vcol16, w99 = [], [], [], []
    for ht in range(HT):
        pv = psB.tile([P, 1], F32, tag="ps")
        nc.tensor.transpose(pv, vrow[0:1, ht * P:(ht + 1) * P], ident[0:1, 0:1])
        t = wpool.tile([P, 1], F32, tag=f"vcolf{ht}")
        nc.vector.tensor_copy(out=t, in_=pv)
        vcolf.append(t)

        pb = psB.tile([P, 1], F32, tag="ps")
        nc.tensor.transpose(pb, brow[0:1, ht * P:(ht + 1) * P], ident[0:1, 0:1])
        tb = wpool.tile([P, 1], F32, tag=f"bcol{ht}")
        nc.vector.tensor_copy(out=tb, in_=pb)
        bcol.append(tb)

        t16 = wpool.tile([P, 1], BF16, tag=f"vcol16{ht}")
        nc.vector.tensor_copy(out=t16, in_=pv)
        vcol16.append(t16)

        # sliding one-hot stationaries for 32-column-group matmuls.
        # we[:, 32] = 0.99*v (even j%32), wo[:, 33] = same (odd j%32): every
        # 32-wide slice starts 4-byte aligned.
        we = wpool.tile([P, 64], BF16, tag=f"w99e{ht}", name=f"w99e{ht}")
        nc.vector.memset(we, 0.0)
        nc.vector.tensor_scalar(out=we[:, 32:33], in0=pv, scalar1=1.0 - SLOPE,
                                scalar2=None, op0=OP.mult)
        wo = wpool.tile([P, 66], BF16, tag=f"w99o{ht}", name=f"w99o{ht}")
        nc.vector.memset(wo, 0.0)
        nc.vector.tensor_scalar(out=wo[:, 33:34], in0=pv, scalar1=1.0 - SLOPE,
                                scalar2=None, op0=OP.mult)
        w99.append((we, wo))

    # ---------------- prep: loads/transposes/projections ----------------
    # Emitted as closures so batch b+1's prep instructions can be interleaved
    # into batch b's first qtile loop (PE executes in program order; a solid
    # prep burst would stall the elementwise engines).
    prep = [dict() for _ in range(BL)]

    def emit_loads(b):
        st = prep[b]
        st["qsb"] = [bpool.tile([P, H], F32, tag=f"qsb{i}", name=f"qsb{i}") for i in range(QT)]
        st["csb"] = [bpool.tile([P, H], F32, tag=f"csb{i}", name=f"csb{i}") for i in range(CT)]
        for qi in range(QT):
            nc.sync.dma_start(out=st["qsb"][qi], in_=q_d[b, qi * P:(qi + 1) * P, :])
        for ci in range(CT):
            nc.sync.dma_start(out=st["csb"][ci], in_=c_d[b, ci * P:(ci + 1) * P, :])
        st["mrow_i"] = bpool.tile([1, L], I32, tag="mrow_i", name="mrow_i")
        nc.sync.dma_start(out=st["mrow_i"], in_=m_d[b:b + 1, :])

    def prep_tasks(b):
        st = prep[b]
        tasks = []

        def t_mask():
            mrow_f = bpool.tile([1, L], F32, tag="mrow_f", name="mrow_f")
            nc.vector.tensor_copy(out=mrow_f, in_=st["mrow_i"])
            maskb = bpool.tile([1, L], F32, tag="maskb", name="maskb")
            nc.vector.tensor_scalar(out=maskb, in0=mrow_f, scalar1=-1.0, scalar2=1e30,
                                    op0=OP.add, op1=OP.mult)
            st["maskb"] = maskb
        tasks.append(t_mask)

        def t_csb16():
            st["csb16"] = [bpool.tile([P, H], BF16, tag=f"csb16{i}", name=f"csb16{i}")
                           for i in range(CT)]
            for ci in range(CT):
                nc.scalar.copy(out=st["csb16"][ci], in_=st["csb"][ci])
        tasks.append(t_csb16)

        st["qT16"] = [bpool.tile([P, L], BF16, tag=f"qT16{i}", name=f"qT16{i}") for i in range(HT)]
        st["cT16"] = [bpool.tile([P, L], BF16, tag=f"cT16{i}", name=f"cT16{i}") for i in range(HT)]

        def mk_tr(srcname, dstname, ti, hi, on_vec):
            def t():
                pst = psB.tile([P, P], F32, tag="ps", name="ps_t")
                nc.tensor.transpose(pst, st[srcname][ti][:, hi * P:(hi + 1) * P], ident)
                if on_vec:
                    nc.vector.tensor_copy(out=st[dstname][hi][:, ti * P:(ti + 1) * P], in_=pst)
                else:
                    nc.scalar.copy(out=st[dstname][hi][:, ti * P:(ti + 1) * P], in_=pst)
            return t
        cpi = 0
        for srcname, dstname in (("qsb", "qT16"), ("csb", "cT16")):
            for ti in range(QT):
                for hi in range(HT):
                    tasks.append(mk_tr(srcname, dstname, ti, hi, cpi % 2 == 0))
                    cpi += 1

        st["qpbT"] = [bpool.tile([P, L], F32, tag=f"qpbT{i}", name=f"qpbT{i}") for i in range(HT)]
        st["cpT16"] = [bpool.tile([P, L], BF16, tag=f"cpT16{i}", name=f"cpT16{i}") for i in range(HT)]

        def mk_proj(ho, is_q):
            def t():
                ps = psB.tile([P, L], F32, tag="ps", name="ps_p")
                wT = wqT16 if is_q else wcT16
                inT = st["qT16"] if is_q else st["cT16"]
                for ki in range(HT):
                    nc.tensor.matmul(ps, wT[ki][:, ho * P:(ho + 1) * P], inT[ki],
                                     start=(ki == 0), stop=(ki == HT - 1))
                if is_q:
                    nc.vector.tensor_scalar(out=st["qpbT"][ho], in0=ps, scalar1=bcol[ho],
                                            scalar2=None, op0=OP.add)
                else:
                    nc.scalar.copy(out=st["cpT16"][ho], in_=ps)
            return t
        for ho in range(HT):
            tasks.append(mk_proj(ho, True))
            tasks.append(mk_proj(ho, False))

        def t_lin():
            pvq = psB.tile([1, L], F32, tag="ps", name="ps_vq")
            for ht in range(HT):
                nc.tensor.matmul(pvq, vcolf[ht], st["qpbT"][ht],
                                 start=(ht == 0), stop=(ht == HT - 1))
            pvc = psB.tile([1, L], F32, tag="ps", name="ps_vc")
            for ht in range(HT):
                nc.tensor.matmul(pvc, vcol16[ht], st["cpT16"][ht],
                                 start=(ht == 0), stop=(ht == HT - 1))
            linq = bpool.tile([1, L], F32, tag="linq", name="linq")
            nc.vector.tensor_scalar(out=linq, in0=pvq, scalar1=SLOPE,
                                    scalar2=None, op0=OP.mult)
            rowvec = bpool.tile([1, L], F32, tag="rowvec", name="rowvec")
            nc.vector.tensor_scalar(out=rowvec, in0=pvc, scalar1=SLOPE,
                                    scalar2=None, op0=OP.mult)
            nc.vector.tensor_add(rowvec, rowvec, st["maskb"])
            st["linq"] = linq
            st["rowvec"] = rowvec
        tasks.append(t_lin)
        return tasks

    emit_loads(0)
    for t in prep_tasks(0):
        t()
    emit_loads(1)
    pending = prep_tasks(1)

    # ---------------- main phase: score/softmax/output ----------------
    opctr = 0
    for b in range(BL):
        while b > 0 and pending:
            pending.pop(0)()
        qpbT = prep[b]["qpbT"]
        cpT16 = prep[b]["cpT16"]
        csb16 = prep[b]["csb16"]
        attnT16 = [bpool.tile([P, L], BF16, tag=f"attnT16{i}", name=f"attnT16{i}") for i in range(CT)]

        for qi in range(QT):
            linq = prep[b]["linq"]
            rowvec = prep[b]["rowvec"]
            sp = psA.tile([P, L], F32, tag="spsum")
            # full-width rank-1 matmuls first: start=True must cover all 128
            # partitions so every row's has_written bit is freshly set.
            nc.tensor.matmul(sp, linq[0:1, qi * P:(qi + 1) * P], ones_row,
                             start=True, stop=False)
            nc.tensor.matmul(sp, ones_row[0:1, 0:P], rowvec,
                             start=False, stop=False)
            for j in range(P):
                if pending and (j % 6 == 5):
                    pending.pop(0)()
                q = qi * P + j
                g, jr = j // 32, j % 32
                for ht in range(HT):
                    x = xpool.tile([P, L], BF16, tag="x")
                    bias_col = qpbT[ht][:, q:q + 1]
                    if opctr % 12 in ACT_PAT:
                        nc.scalar.activation(out=x, in_=cpT16[ht], func=AF.Relu,
                                             bias=bias_col, scale=1.0)
                    else:
                        nc.vector.tensor_scalar(out=x, in0=cpT16[ht], scalar1=bias_col,
                                                scalar2=0.0, op0=OP.add, op1=OP.max)
                    opctr += 1
                    last = (j == P - 1 and ht == HT - 1)
                    we, wo = w99[ht]
                    lhsT = we[:, 32 - jr:64 - jr] if jr % 2 == 0 else wo[:, 33 - jr:65 - jr]
                    nc.tensor.matmul(sp[32 * g:32 * (g + 1), :], lhsT, x,
                                     start=False, stop=last,
                                     tile_position=(0, 32 * g),
                                     skip_group_check=True)

            # softmax over c
            mx = spool.tile([P, 1], F32, tag="mx")
            nc.vector.tensor_reduce(out=mx, in_=sp, axis=AX.X, op=OP.max, negate=True)
            pexp = spool.tile([P, L], F32, tag="pexp")
            rsum = spool.tile([P, 1], F32, tag="rsum")
            nc.scalar.activation(out=pexp, in_=sp, func=AF.Exp, bias=mx, scale=1.0,
                                 accum_out=rsum)
            rinv = spool.tile([P, 1], F32, tag="rinv")
            nc.vector.reciprocal(out=rinv, in_=rsum)
            attn_sb = spool.tile([P, L], F32, tag="attn_sb")
            nc.vector.tensor_scalar(out=attn_sb, in0=pexp, scalar1=rinv,
                                    scalar2=None, op0=OP.mult)
            nc.sync.dma_start(out=at_d[b, qi * P:(qi + 1) * P, :], in_=attn_sb)

            # attn_output = attn @ context (bf16), transposes on PE
            for ci in range(CT):
                pst = psB.tile([P, P], F32, tag="ps", name="ps_t")
                nc.tensor.transpose(pst, attn_sb[:, ci * P:(ci + 1) * P], ident)
                nc.scalar.copy(out=attnT16[ci][:, qi * P:(qi + 1) * P], in_=pst)
            po = psB.tile([P, H], F32, tag="ps")
            for ci in range(CT):
                nc.tensor.matmul(po, attnT16[ci][:, qi * P:(qi + 1) * P], csb16[ci],
                                 start=(ci == 0), stop=(ci == CT - 1))
            osb = spool.tile([P, H], F32, tag="osb")
            nc.scalar.copy(out=osb, in_=po)
            nc.sync.dma_start(out=ao_d[b, qi * P:(qi + 1) * P, :], in_=osb)


_NC_CACHE = {}


def build_nc():
    if "nc" in _NC_CACHE:
        return _NC_CACHE["nc"]
    nc = bacc.Bacc("TRN2", target_bir_lowering=False)
    with ExitStack() as ctx:
        tc = ctx.enter_context(tile.TileContext(nc))
        _build_body(ctx, tc)
    nc.compile()
    _NC_CACHE["nc"] = nc
    return nc


def kernel(query, context, mask, w_weight, w_bias, score_weight, _trace=False):
    query = np.ascontiguousarray(np.asarray(query, dtype=np.float32))
    context = np.ascontiguousarray(np.asarray(context, dtype=np.float32))
    mask = np.ascontiguousarray(np.asarray(mask, dtype=np.int32))
    w_weight = np.ascontiguousarray(np.asarray(w_weight, dtype=np.float32))
    w_bias = np.ascontiguousarray(np.asarray(w_bias, dtype=np.float32))
    score_weight = np.ascontiguousarray(np.asarray(score_weight, dtype=np.float32))

    nc = build_nc()
    in_maps = []
    for i in range(NCORES):
        sl = slice(i * BL, (i + 1) * BL)
        in_maps.append({
            "query": query[sl], "context": context[sl], "mask": mask[sl],
            "w_weight": w_weight, "w_bias": w_bias, "score_weight": score_weight,
        })
    res = run_bass_kernel_spmd(nc, in_maps, core_ids=list(range(NCORES)),
                               trace=_trace)
    attn_output = np.concatenate([r["attn_output"] for r in res.results], axis=0)
    attn = np.concatenate([r["attn"] for r in res.results], axis=0)
    if _trace:
        kernel.last_exec_time_ns = res.exec_time_ns
        kernel.last_results = res
    return attn_output, attn
